# revision 2
# baseline (speedup 1.0000x reference)
"""Anisotropic Chebyshev graph convolution on 8 Trainium2 NeuronCores — v4.1.

  out[b,u,m,n] = sum_{k,l,i,p,q} coefs[k,l,i,u] cheb1[k,p,m] cheb2[l,q,n] x[b,i,p,q]

Sharding: data-parallel over batch B=8, one sample per core; cheb1/cheb2/coefs
replicated (no collectives). Per core, three matmul stages in bf16 (fp32 psum):

  1) a[k,i,m,q]   = sum_p cheb1[k,p,m] x[i,p,q]          (contract p)
  2) c[l,u,m,q]   = sum_{ki} W[ki,lu] a[ki,m,q]          (contract k*C = 160)
  3) out[u,m,n]   = sum_{l,q} c[l,u,m,q] cheb2[l,q,n]    (contract q, acc l)

The m->ki transpose of `a` must round-trip DRAM (SBUF DMA access patterns
cannot cross partitions — interior-partition APs are rejected by the BIR
verifier or, worse, produce wrong descriptors). With all 8 cores in
lockstep the bounce makes the kernel HBM-bound on the shared per-pair HBM
stack, so v4 (vs v2) focuses on lowering and smoothing HBM demand:
  - output stored bf16 [m, u, n] (host transposes + widens): -4MB/core
  - each k's piece-readbacks are issued right after that k's bounce write
    (instead of all at once at stage-2 start), spreading the 21MB of read
    demand across the whole timeline and letting stage 2 start earlier
  - a-piece buffers (3-deep) and tail buffers (2-deep) cycle through SBUF,
    readbacks ride three different queues
  - startup loads split finer (first matmul's operands land first)
"""

import numpy as np

import concourse.bacc as bacc
import concourse.bass as bass
import concourse.mybir as mybir
import concourse.tile as tile
from concourse import bass_utils

B = 8
C = 32          # input channels i
U = 32          # output units u
K = 5           # chebyshev powers (k and l)
N1 = 256        # first graph axis (p -> m)
N2 = 256        # second graph axis (q -> n)
P = 128

KI = K * C              # 160 mix contraction
LU = K * U              # 160 mix output
N_CORES = 8

F32 = mybir.dt.float32
BF16 = mybir.dt.bfloat16

MB = 64                 # m's per a-piece
GPP = MB // 4           # 4-m groups per piece
CHUNK = 512
NCHUNK = (C * N2) // CHUNK   # 16


def build(n_iters=1):
    nc = bacc.Bacc("TRN2", target_bir_lowering=False, debug=False, num_devices=1)

    x_d = nc.dram_tensor("x", [C, N1, N2], BF16, kind="ExternalInput")
    ch1_d = nc.dram_tensor("cheb1", [K, N1, N1], BF16, kind="ExternalInput")
    ch2_d = nc.dram_tensor("cheb2", [K, N2, N2], BF16, kind="ExternalInput")
    w1_d = nc.dram_tensor("w1", [P, LU], BF16, kind="ExternalInput")
    # w2rep[32j+t, :] = W[128+t, :] for j in 0..4 (tail weights at each strip)
    w2_d = nc.dram_tensor("w2rep", [P, LU], BF16, kind="ExternalInput")
    out_d = nc.dram_tensor("out", [N1, U, N2], BF16, kind="ExternalOutput")

    with tile.TileContext(nc) as tc:
      for _it in range(n_iters):
        with (
            tc.tile_pool(name="const", bufs=1) as const_pool,
            tc.tile_pool(name="adram", bufs=1, space="DRAM") as dram_pool,
            tc.tile_pool(name="am", bufs=3) as am_pool,
            tc.tile_pool(name="at", bufs=2) as at_pool,
            tc.tile_pool(name="csb", bufs=8) as c_pool,
            tc.tile_pool(name="osb", bufs=6) as o_pool,
        ):
            # x as [p, (ph), i*q]
            xp = [
                const_pool.tile([P, C * N2], BF16, tag=f"xp{ph}", name=f"xp{ph}")
                for ph in range(2)
            ]
            xv = [xp[ph][:].rearrange("p (i q) -> p i q", q=N2) for ph in range(2)]
            x_r = x_d.ap().rearrange("i (ph p) q -> ph p i q", ph=2)
            ch1 = const_pool.tile([P, K, 2, N1], BF16, tag="ch1")
            ch1_r = ch1_d.ap().rearrange("k (ph p) m -> p k ph m", ph=2)
            # k=0 slice first so the first matmul's stationary lands early
            nc.sync.dma_start(ch1[:, 0:1], ch1_r[:, 0:1])
            nc.sync.dma_start(xv[0][:, :4], x_r[0, :, :4])
            nc.scalar.dma_start(xv[1][:, :4], x_r[1, :, :4])
            nc.sync.dma_start(xv[0][:, 4:16], x_r[0, :, 4:16])
            nc.scalar.dma_start(xv[1][:, 4:16], x_r[1, :, 4:16])
            nc.sync.dma_start(xv[0][:, 16:], x_r[0, :, 16:])
            nc.gpsimd.dma_start(xv[1][:, 16:], x_r[1, :, 16:])
            nc.sync.dma_start(ch1[:, 1:], ch1_r[:, 1:])
            ch2 = const_pool.tile([P, K, 2, N2], BF16, tag="ch2")
            nc.scalar.dma_start(
                ch2[:], ch2_d.ap().rearrange("l (qh q) n -> q l qh n", qh=2)
            )
            w1 = const_pool.tile([P, LU], BF16, tag="w1")
            nc.scalar.dma_start(w1[:], w1_d.ap())
            w2r = const_pool.tile([P, LU], BF16, tag="w2r")
            nc.scalar.dma_start(w2r[:], w2_d.ap())

            # `a` bounce in DRAM, stored transposed [ki, m, q] per m-half
            aT = [
                dram_pool.tile([KI, P, N2], BF16, tag=f"aT{mh}", name=f"aT{mh}")
                for mh in range(2)
            ]

            am_tiles = [None] * 4    # [ki(32k+i), m_local, q] per 64-m piece
            at_tiles = [None] * 4    # [32*(m%4)+i, m_local//4, q]
            evac_flip = 0

            # ---- stage 1 + k-split readbacks -----------------------------
            with (
                tc.tile_pool(name="ps_a", bufs=2, space="PSUM") as ps_a,
                tc.tile_pool(name="ae", bufs=2) as ae_pool,
            ):
                for mh in range(2):
                    for half in range(2):
                        pc = 2 * mh + half
                        am_tiles[pc] = am_pool.tile(
                            [P, MB, N2], BF16, tag="am", name=f"am{pc}_{_it}"
                        )
                        at_tiles[pc] = at_pool.tile(
                            [P, GPP, N2], BF16, tag="at", name=f"at{pc}_{_it}"
                        )
                    for k in range(K):
                        ae = ae_pool.tile([P, C, N2], BF16, tag="ae")
                        for cg in range(NCHUNK // 4):
                            ps = ps_a.tile([P, 4 * CHUNK], F32, tag="ps_a")
                            for ph in range(2):
                                lhsT = ch1[:, k, ph, mh * P : (mh + 1) * P]
                                for cj in range(4):
                                    ci = cg * 4 + cj
                                    nc.tensor.matmul(
                                        ps[:, cj * CHUNK : (cj + 1) * CHUNK],
                                        lhsT,
                                        xp[ph][:, ci * CHUNK : (ci + 1) * CHUNK],
                                        start=(ph == 0),
                                        stop=(ph == 1),
                                    )
                            dst = ae[:, 8 * cg : 8 * cg + 8, :]
                            if evac_flip == 0:
                                nc.vector.tensor_copy(dst, ps[:])
                            else:
                                nc.scalar.copy(dst, ps[:])
                            evac_flip ^= 1
                        # bounce write (transposed on the DRAM-side AP), then
                        # immediately read this k's rows back into the piece
                        # buffers: half0 on the writing queue (FIFO-ordered),
                        # half1 on gpsimd
                        weng = nc.scalar if k % 2 else nc.sync
                        if k < 4:
                            dst_ap = aT[mh][k * C : (k + 1) * C, :, :].rearrange(
                                "i m q -> m i q"
                            )
                            weng.dma_start(dst_ap, ae[:])
                            for half in range(2):
                                pc = 2 * mh + half
                                reng = nc.gpsimd if half else weng
                                reng.dma_start(
                                    am_tiles[pc][32 * k : 32 * (k + 1)],
                                    aT[mh][
                                        k * C : (k + 1) * C,
                                        half * MB : (half + 1) * MB,
                                        :,
                                    ],
                                )
                        else:
                            dst_ap = aT[mh][P:KI, :, :].rearrange("i m q -> m i q")
                            weng.dma_start(dst_ap, ae[:])
                            for half in range(2):
                                pc = 2 * mh + half
                                reng = nc.gpsimd if half else weng
                                for j in range(4):
                                    src = aT[mh][
                                        P:KI, half * MB + j : (half + 1) * MB : 4, :
                                    ]
                                    reng.dma_start(
                                        at_tiles[pc][32 * j : 32 * (j + 1)], src
                                    )

            # ---- stages 2+3 per 64-m piece -------------------------------
            with (
                tc.tile_pool(name="ps_c", bufs=6, space="PSUM") as ps_c,
                tc.tile_pool(name="ps_o", bufs=2, space="PSUM") as ps_o,
            ):
                for pc in range(4):
                    am_t, at_t = am_tiles[pc], at_tiles[pc]
                    for g in range(GPP):
                        c_tiles = []
                        for qh in range(2):
                            csb = c_pool.tile([P, K, 4, U], BF16, tag="csb")
                            c_tiles.append(csb)
                            cps_l = []
                            for mj in range(4):
                                cps = ps_c.tile([P, LU], F32, tag="ps_c")
                                cps_l.append(cps)
                                nc.tensor.matmul(
                                    cps[:],
                                    am_t[:, g * 4 + mj, qh * P : (qh + 1) * P],
                                    w1[:],
                                    start=True,
                                    stop=False,
                                )
                            for mj in range(4):
                                nc.tensor.matmul(
                                    cps_l[mj][:],
                                    at_t[32 * mj : 32 * mj + 32, g, qh * P : (qh + 1) * P],
                                    w2r[32 * mj : 32 * mj + 32, :],
                                    start=False,
                                    stop=True,
                                    tile_position=(32 * mj, 0),
                                )
                            for mj in range(4):
                                src = cps_l[mj][:].rearrange("p (l u) -> p l u", u=U)
                                dst = csb[:, :, mj, :]
                                if evac_flip == 0:
                                    nc.vector.tensor_copy(dst, src)
                                else:
                                    nc.scalar.copy(dst, src)
                                evac_flip ^= 1

                        # stage 3: out[(mj,u), n] += c^T cheb2
                        ops = ps_o.tile([P, N2], F32, tag="ps_o")
                        for l in range(K):
                            for qh in range(2):
                                nc.tensor.matmul(
                                    ops[:],
                                    c_tiles[qh][:, l],
                                    ch2[:, l, qh, :],
                                    start=(l == 0 and qh == 0),
                                    stop=(l == K - 1 and qh == 1),
                                )
                        osb = o_pool.tile([P, N2], BF16, tag="osb")
                        if evac_flip == 0:
                            nc.vector.tensor_copy(osb[:], ops[:])
                        else:
                            nc.scalar.copy(osb[:], ops[:])
                        evac_flip ^= 1
                        m_abs = pc * MB + g * 4
                        dst = out_d.ap()[m_abs : m_abs + 4, :, :].rearrange(
                            "m u n -> (m u) n"
                        )
                        oeng = nc.scalar if g % 2 else nc.sync
                        oeng.dma_start(dst, osb[:])

    nc.compile()
    return nc


_NC = None
LAST_RUN = {}


def _bf16(a):
    import ml_dtypes

    return np.asarray(a, dtype=np.float32).astype(ml_dtypes.bfloat16)


def _weights(coefs):
    w = np.asarray(coefs, np.float32).transpose(0, 2, 1, 3).reshape(KI, LU)
    w1 = w[:P]
    w2rep = np.tile(w[P:KI], (4, 1))
    return _bf16(np.ascontiguousarray(w1)), _bf16(np.ascontiguousarray(w2rep))


def core_input_map(x, cheb1, cheb2, coefs, core):
    w1, w2rep = _weights(coefs)
    return {
        "x": _bf16(np.asarray(x, np.float32)[core]),
        "cheb1": _bf16(cheb1),
        "cheb2": _bf16(cheb2),
        "w1": w1,
        "w2rep": w2rep,
    }


def core_expected(expected, core):
    return np.asarray(expected)[core].transpose(1, 0, 2)  # [u,m,n] -> [m,u,n]


def kernel(x, cheb1, cheb2, coefs):
    global _NC
    import time as _time

    if _NC is None:
        t0 = _time.monotonic()
        _NC = build()
        LAST_RUN["build_s"] = _time.monotonic() - t0

    w1, w2rep = _weights(coefs)
    ch1b, ch2b = _bf16(cheb1), _bf16(cheb2)
    xb = _bf16(x)

    in_maps = [
        {"x": xb[b], "cheb1": ch1b, "cheb2": ch2b, "w1": w1, "w2rep": w2rep}
        for b in range(B)
    ]

    t0 = _time.monotonic()
    res = bass_utils.run_bass_kernel_spmd(_NC, in_maps, core_ids=list(range(N_CORES)))
    LAST_RUN["wall_s"] = _time.monotonic() - t0
    LAST_RUN["exec_time_ns"] = res.exec_time_ns

    # out is [m, u, n] bf16 per core -> [u, m, n] f32
    return np.stack(
        [
            np.asarray(res.results[b]["out"], dtype=np.float32).transpose(1, 0, 2)
            for b in range(B)
        ]
    )


# revision 5
# speedup vs baseline: 1.0303x; 1.0303x over previous
"""Anisotropic Chebyshev graph convolution on 8 Trainium2 NeuronCores — v4.

  out[b,u,m,n] = sum_{k,l,i,p,q} coefs[k,l,i,u] cheb1[k,p,m] cheb2[l,q,n] x[b,i,p,q]

Sharding: data-parallel over batch B=8, one sample per core; cheb1/cheb2/coefs
replicated (no collectives). Per core, three matmul stages in bf16 (fp32 psum):

  1) a[k,i,m,q]   = sum_p cheb1[k,p,m] x[i,p,q]          (contract p)
  2) c[l,u,m,q]   = sum_{ki} W[ki,lu] a[ki,m,q]          (contract k*C = 160)
  3) out[u,m,n]   = sum_{l,q} c[l,u,m,q] cheb2[l,q,n]    (contract q, acc l)

The m->ki transpose of `a` must round-trip DRAM (SBUF DMA access patterns
cannot cross partitions — interior-partition APs are rejected by the BIR
verifier or, worse, produce wrong descriptors). With all 8 cores in
lockstep the bounce makes the kernel HBM-bound on the shared per-pair HBM
stack, so v4 (vs v2) focuses on lowering and smoothing HBM demand:
  - output stored bf16 [m, u, n] (host transposes + widens): -4MB/core
  - each k's piece-readbacks are issued right after that k's bounce write
    (instead of all at once at stage-2 start), spreading the 21MB of read
    demand across the whole timeline and letting stage 2 start earlier
  - a-piece buffers (3-deep) and tail buffers (2-deep) cycle through SBUF,
    readbacks ride three different queues
  - startup loads split finer (first matmul's operands land first)
"""

import numpy as np

import concourse.bacc as bacc
import concourse.bass as bass
import concourse.mybir as mybir
import concourse.tile as tile
from concourse import bass_utils

B = 8
C = 32          # input channels i
U = 32          # output units u
K = 5           # chebyshev powers (k and l)
N1 = 256        # first graph axis (p -> m)
N2 = 256        # second graph axis (q -> n)
P = 128

KI = K * C              # 160 mix contraction
LU = K * U              # 160 mix output
N_CORES = 8

F32 = mybir.dt.float32
BF16 = mybir.dt.bfloat16

MB = 64                 # m's per a-piece
GPP = MB // 4           # 4-m groups per piece
CHUNK = 512
NCHUNK = (C * N2) // CHUNK   # 16


def build(n_iters=1):
    nc = bacc.Bacc("TRN2", target_bir_lowering=False, debug=False, num_devices=1)

    x_d = nc.dram_tensor("x", [C, N1, N2], BF16, kind="ExternalInput")
    ch1_d = nc.dram_tensor("cheb1", [K, N1, N1], BF16, kind="ExternalInput")
    ch2_d = nc.dram_tensor("cheb2", [K, N2, N2], BF16, kind="ExternalInput")
    w1_d = nc.dram_tensor("w1", [P, LU], BF16, kind="ExternalInput")
    # w2rep[32j+t, :] = W[128+t, :] for j in 0..4 (tail weights at each strip)
    w2_d = nc.dram_tensor("w2rep", [P, LU], BF16, kind="ExternalInput")
    out_d = nc.dram_tensor("out", [N1, U, N2], BF16, kind="ExternalOutput")

    with tile.TileContext(nc) as tc:
      for _it in range(n_iters):
        with (
            tc.tile_pool(name="const", bufs=1) as const_pool,
            tc.tile_pool(name="adram", bufs=1, space="DRAM") as dram_pool,
            tc.tile_pool(name="am", bufs=3) as am_pool,
            tc.tile_pool(name="at", bufs=2) as at_pool,
            tc.tile_pool(name="csb", bufs=12) as c_pool,
            tc.tile_pool(name="osb", bufs=8) as o_pool,
        ):
            # x as [p, (ph), i*q]
            xp = [
                const_pool.tile([P, C * N2], BF16, tag=f"xp{ph}", name=f"xp{ph}")
                for ph in range(2)
            ]
            xv = [xp[ph][:].rearrange("p (i q) -> p i q", q=N2) for ph in range(2)]
            x_r = x_d.ap().rearrange("i (ph p) q -> ph p i q", ph=2)
            ch1 = const_pool.tile([P, K, 2, N1], BF16, tag="ch1")
            ch1_r = ch1_d.ap().rearrange("k (ph p) m -> p k ph m", ph=2)
            # k=0 slice first so the first matmul's stationary lands early
            nc.sync.dma_start(ch1[:, 0:1], ch1_r[:, 0:1])
            nc.sync.dma_start(xv[0][:, :4], x_r[0, :, :4])
            nc.scalar.dma_start(xv[1][:, :4], x_r[1, :, :4])
            nc.sync.dma_start(xv[0][:, 4:16], x_r[0, :, 4:16])
            nc.scalar.dma_start(xv[1][:, 4:16], x_r[1, :, 4:16])
            nc.sync.dma_start(xv[0][:, 16:], x_r[0, :, 16:])
            nc.gpsimd.dma_start(xv[1][:, 16:], x_r[1, :, 16:])
            nc.sync.dma_start(ch1[:, 1:], ch1_r[:, 1:])
            ch2 = const_pool.tile([P, K, 2, N2], BF16, tag="ch2")
            nc.scalar.dma_start(
                ch2[:], ch2_d.ap().rearrange("l (qh q) n -> q l qh n", qh=2)
            )
            w1 = const_pool.tile([P, LU], BF16, tag="w1")
            nc.scalar.dma_start(w1[:], w1_d.ap())
            w2r = const_pool.tile([P, LU], BF16, tag="w2r")
            nc.scalar.dma_start(w2r[:], w2_d.ap())

            # `a` bounce in DRAM, stored transposed [ki, m, q] per m-half
            aT = [
                dram_pool.tile([KI, P, N2], BF16, tag=f"aT{mh}", name=f"aT{mh}")
                for mh in range(2)
            ]

            am_tiles = [None] * 4    # [ki(32k+i), m_local, q] per 64-m piece
            at_tiles = [None] * 4    # [32*(m%4)+i, m_local//4, q]
            evac_flip = 0

            # ---- stage 1 + k-split readbacks -----------------------------
            with (
                tc.tile_pool(name="ps_a", bufs=2, space="PSUM") as ps_a,
                tc.tile_pool(name="ae", bufs=2) as ae_pool,
            ):
                for mh in range(2):
                    for half in range(2):
                        pc = 2 * mh + half
                        am_tiles[pc] = am_pool.tile(
                            [P, MB, N2], BF16, tag="am", name=f"am{pc}_{_it}"
                        )
                        at_tiles[pc] = at_pool.tile(
                            [P, GPP, N2], BF16, tag="at", name=f"at{pc}_{_it}"
                        )
                    for k in range(K):
                        ae = ae_pool.tile([P, C, N2], BF16, tag="ae")
                        for cg in range(NCHUNK // 4):
                            ps = ps_a.tile([P, 4 * CHUNK], F32, tag="ps_a")
                            for ph in range(2):
                                lhsT = ch1[:, k, ph, mh * P : (mh + 1) * P]
                                for cj in range(4):
                                    ci = cg * 4 + cj
                                    nc.tensor.matmul(
                                        ps[:, cj * CHUNK : (cj + 1) * CHUNK],
                                        lhsT,
                                        xp[ph][:, ci * CHUNK : (ci + 1) * CHUNK],
                                        start=(ph == 0),
                                        stop=(ph == 1),
                                    )
                            dst = ae[:, 8 * cg : 8 * cg + 8, :]
                            if evac_flip == 0:
                                nc.vector.tensor_copy(dst, ps[:])
                            else:
                                nc.scalar.copy(dst, ps[:])
                            evac_flip ^= 1
                        # bounce write (transposed on the DRAM-side AP), then
                        # immediately read this k's rows back into the piece
                        # buffers: half0 on the writing queue (FIFO-ordered),
                        # half1 on gpsimd
                        weng = nc.scalar if k % 2 else nc.sync
                        if k < 4:
                            dst_ap = aT[mh][k * C : (k + 1) * C, :, :].rearrange(
                                "i m q -> m i q"
                            )
                            weng.dma_start(dst_ap, ae[:])
                            for half in range(2):
                                pc = 2 * mh + half
                                reng = nc.gpsimd if half else weng
                                reng.dma_start(
                                    am_tiles[pc][32 * k : 32 * (k + 1)],
                                    aT[mh][
                                        k * C : (k + 1) * C,
                                        half * MB : (half + 1) * MB,
                                        :,
                                    ],
                                )
                        else:
                            dst_ap = aT[mh][P:KI, :, :].rearrange("i m q -> m i q")
                            weng.dma_start(dst_ap, ae[:])
                            for half in range(2):
                                pc = 2 * mh + half
                                reng = nc.gpsimd if half else weng
                                for j in range(4):
                                    src = aT[mh][
                                        P:KI, half * MB + j : (half + 1) * MB : 4, :
                                    ]
                                    reng.dma_start(
                                        at_tiles[pc][32 * j : 32 * (j + 1)], src
                                    )

            # ---- stages 2+3 per 64-m piece -------------------------------
            with (
                tc.tile_pool(name="ps_c", bufs=6, space="PSUM") as ps_c,
                tc.tile_pool(name="ps_o", bufs=2, space="PSUM") as ps_o,
            ):
                for pc in range(4):
                    am_t, at_t = am_tiles[pc], at_tiles[pc]
                    for g in range(GPP):
                        c_tiles = []
                        for qh in range(2):
                            csb = c_pool.tile([P, K, 4, U], BF16, tag="csb")
                            c_tiles.append(csb)
                            cps_l = []
                            for mj in range(4):
                                cps = ps_c.tile([P, LU], F32, tag="ps_c")
                                cps_l.append(cps)
                                nc.tensor.matmul(
                                    cps[:],
                                    am_t[:, g * 4 + mj, qh * P : (qh + 1) * P],
                                    w1[:],
                                    start=True,
                                    stop=False,
                                )
                            for mj in range(4):
                                nc.tensor.matmul(
                                    cps_l[mj][:],
                                    at_t[32 * mj : 32 * mj + 32, g, qh * P : (qh + 1) * P],
                                    w2r[32 * mj : 32 * mj + 32, :],
                                    start=False,
                                    stop=True,
                                    tile_position=(32 * mj, 0),
                                )
                            for mj in range(4):
                                src = cps_l[mj][:].rearrange("p (l u) -> p l u", u=U)
                                dst = csb[:, :, mj, :]
                                if mj == 3 - evac_flip:
                                    nc.scalar.copy(dst, src)
                                else:
                                    nc.vector.tensor_copy(dst, src)
                            evac_flip ^= 1

                        # stage 3: out[(mj,u), n] += c^T cheb2
                        # (qh-major so the qh0 accumulation overlaps stage-2's
                        # qh1 evacuation on the PE stream)
                        ops = ps_o.tile([P, N2], F32, tag="ps_o")
                        for qh in range(2):
                            for l in range(K):
                                nc.tensor.matmul(
                                    ops[:],
                                    c_tiles[qh][:, l],
                                    ch2[:, l, qh, :],
                                    start=(l == 0 and qh == 0),
                                    stop=(l == K - 1 and qh == 1),
                                )
                        osb = o_pool.tile([P, N2], BF16, tag="osb")
                        if evac_flip == 0:
                            nc.vector.tensor_copy(osb[:], ops[:])
                        else:
                            nc.scalar.copy(osb[:], ops[:])
                        evac_flip ^= 1
                        m_abs = pc * MB + g * 4
                        dst = out_d.ap()[m_abs : m_abs + 4, :, :].rearrange(
                            "m u n -> (m u) n"
                        )
                        oeng = nc.scalar if g % 2 else nc.sync
                        oeng.dma_start(dst, osb[:])

    nc.compile()
    return nc


_NC = None
LAST_RUN = {}


def _bf16(a):
    import ml_dtypes

    return np.asarray(a, dtype=np.float32).astype(ml_dtypes.bfloat16)


def _weights(coefs):
    w = np.asarray(coefs, np.float32).transpose(0, 2, 1, 3).reshape(KI, LU)
    w1 = w[:P]
    w2rep = np.tile(w[P:KI], (4, 1))
    return _bf16(np.ascontiguousarray(w1)), _bf16(np.ascontiguousarray(w2rep))


def core_input_map(x, cheb1, cheb2, coefs, core):
    w1, w2rep = _weights(coefs)
    return {
        "x": _bf16(np.asarray(x, np.float32)[core]),
        "cheb1": _bf16(cheb1),
        "cheb2": _bf16(cheb2),
        "w1": w1,
        "w2rep": w2rep,
    }


def core_expected(expected, core):
    return np.asarray(expected)[core].transpose(1, 0, 2)  # [u,m,n] -> [m,u,n]


def kernel(x, cheb1, cheb2, coefs):
    global _NC
    import time as _time

    if _NC is None:
        t0 = _time.monotonic()
        _NC = build()
        LAST_RUN["build_s"] = _time.monotonic() - t0

    w1, w2rep = _weights(coefs)
    ch1b, ch2b = _bf16(cheb1), _bf16(cheb2)
    xb = _bf16(x)

    in_maps = [
        {"x": xb[b], "cheb1": ch1b, "cheb2": ch2b, "w1": w1, "w2rep": w2rep}
        for b in range(B)
    ]

    t0 = _time.monotonic()
    res = bass_utils.run_bass_kernel_spmd(_NC, in_maps, core_ids=list(range(N_CORES)))
    LAST_RUN["wall_s"] = _time.monotonic() - t0
    LAST_RUN["exec_time_ns"] = res.exec_time_ns

    # out is [m, u, n] bf16 per core -> [u, m, n] f32
    return np.stack(
        [
            np.asarray(res.results[b]["out"], dtype=np.float32).transpose(1, 0, 2)
            for b in range(B)
        ]
    )
